# revision 10
# baseline (speedup 1.0000x reference)
"""Trainium2 Bass kernel for nn_Block2x2DiagProduct (butterfly product).

Strategy:
  Stages 1..9 of the butterfly (all with block size <= 512) compose into
  blockdiag(R, R) with a single dense 512x512 matrix R shared by both
  halves (parameters are shared across blocks within each factor). The
  final stage (block size 1024) is a columnwise 2x2 butterfly:

      out[:, k]     = A[k]*y[:, k] + B[k]*y[:, 512+k]
      out[:, 512+k] = C[k]*y[:, k] + D[k]*y[:, 512+k]

  where y = x @ blockdiag(R^T, R^T). So the device kernel is two K=512
  float32r matmuls per row tile (PE) plus six columnwise multiply/adds
  (Vector + GpSimd), with Scalar staging the PE-transposed x to SBUF.

  v2 changes (trace-driven; steady state was PE-bound at 100% busy with
  DMA engines ~90%):
    - x is declared float32r end-to-end: PE transpose-mode streams f32r
      at 1.5 cycles/row vs plain fp32's 2.0 (bit-identical storage).
    - Row packing "(p j) f -> p j f": each partition holds 4 consecutive
      HBM rows, so load descriptors are 16 KiB and store descriptors
      8 KiB (vs 4 KiB), lifting per-DMA-engine throughput.
    - Stage-0 coefficients: 8 KB DMA to partition 0 + on-chip GpSimd
      partition_broadcast (was a host-broadcast 1 MiB DMA).
    - Software-pipelined PE stream: transposes of tile t+1 are emitted
      before the matmuls of tile t, so PE never waits on the Scalar
      PSUM->SBUF staging copies.
    - Matmuls grouped h-outer so the first accumulation group only waits
      on its own half's staged transpose.

  R is composed on the host in float64 (9 einsums over a 512x512
  identity). Sharding: pure data parallel - batch dim of x split across
  8 cores; R^T (1 MiB) and the stage-0 coefficients are replicated.
"""

import os
import sys

for _p in ("/opt/trn_rl_repo", "/root/.axon_site/_ro/trn_rl_repo"):
    if os.path.isdir(_p) and _p not in sys.path:
        sys.path.insert(0, _p)

import numpy as np

import concourse.bacc as bacc
import concourse.bass as bass
import concourse.mybir as mybir
from concourse.bass_utils import run_bass_kernel_spmd
from concourse.masks import make_identity
from concourse.tile import TileContext

SIZE = 1024
HALF = SIZE // 2
M = 10  # number of butterfly factors
N_CORES = 8
P = 128
KC = HALF // P  # 4 contraction chunks per half
J = 2  # rows per partition per block (8 KiB contiguous HBM per partition)
BLK = P * J  # 512 rows per block

# Results of the last device run (for the test harness).
last_exec_time_ns = None
last_mean_exec_time_ns = None

_nc_cache = {}


def _compose_w1t(params):
    """Compose butterfly stages 1..9 into W1t (512x512, f64) such that
    y_half = x_half @ W1t for each 512 half. Both halves share W1t because
    each factor's parameters are shared across its blocks."""
    w = np.eye(HALF, dtype=np.float64)
    for i in reversed(range(1, M)):
        s = SIZE >> i
        y = w.reshape(HALF, HALF // s, 2, s // 2)
        w = np.einsum(
            "ijk,bnjk->bnik", params[i].astype(np.float64), y
        ).reshape(HALF, HALF)
    return w


def _build_nc(rows):
    f32 = mybir.dt.float32
    f32r = mybir.dt.float32r
    nblk = rows // BLK

    # Bacc (not raw Bass): its finalize() pipeline splits multi-sem waits
    # into EventSemaphore instructions (HW allows 1 sync-wait per inst).
    nc = bacc.Bacc(None, target_bir_lowering=False)
    x_d = nc.dram_tensor("x", [rows, SIZE], f32r, kind="ExternalInput")
    w_d = nc.dram_tensor("w", [HALF, HALF], f32, kind="ExternalInput")
    coef_d = nc.dram_tensor("coef", [P, 4, HALF], f32, kind="ExternalInput")
    o_d = nc.dram_tensor("o", [rows, SIZE], f32, kind="ExternalOutput")

    with TileContext(nc) as tc:
        with (
            tc.tile_pool(name="const", bufs=1) as const_pool,
            tc.tile_pool(name="xin", bufs=6) as xpool,
            tc.tile_pool(name="xt", bufs=8) as xtpool,
            tc.tile_pool(name="stage", bufs=6) as spool,
            tc.tile_pool(name="osb", bufs=3) as opool,
            tc.tile_pool(name="tpsum", bufs=4, space="PSUM") as tpsum,
            tc.tile_pool(name="mpsum", bufs=4, space="PSUM") as mpsum,
        ):
            ident_f32 = const_pool.tile([P, P], f32)
            make_identity(nc, ident_f32[:])
            # GpSimd memset can't target f32r tiles, so build in f32 and
            # cast (f32r transpose needs an f32r identity operand).
            ident = const_pool.tile([P, P], f32r)
            nc.vector.tensor_copy(out=ident[:], in_=ident_f32[:])
            # PE warmup burst: the PE HAM clock-gate defaults to 1.2 GHz
            # and needs ~3.4us of sustained busy to release to 2.4 GHz.
            # The PE would otherwise sit idle until the first x load lands
            # (~12us) and then run the first blocks at half clock. These
            # no-dependency matmuls (first one doubles as the dummy
            # consuming the identity, which walrus needs so the first real
            # transpose carries a single sync-wait) run during the load
            # window and cost nothing.
            pst0 = tpsum.tile([P, P], f32r, name="pst_warm", tag="pst")
            for _ in range(36):
                nc.tensor.transpose(pst0[:], ident[:], ident[:])

            # Stage-0 coefficients A,B,C,D, pre-replicated across partitions
            # on the host (1 MiB). This DMA goes FIRST on the ACT queue:
            # every Vector multiply waits on it, and a late coef load was
            # observed stalling the whole stage-0 chain (and with it PSUM
            # recycling) until ~30us. A GpSimd partition_broadcast of an
            # 8 KB load was tried instead and was worse: the single-partition
            # DMA itself straggled to ~17us and the broadcast forced two
            # GpSimd library swaps that blocked the stage-0 adds.
            coef_sb = const_pool.tile([P, 4, HALF], f32)
            nc.scalar.dma_start(out=coef_sb[:], in_=coef_d[:, :, :])
            # W1t resident in SBUF: partition p, chunk c holds W1t[c*128+p, :].
            # ACT HWDGE queue + per-chunk loads: doesn't serialize the x loads
            # on the SP queue, and chunk 0's float32r cast is ready early.
            w_sb = const_pool.tile([P, KC, HALF], f32)
            w_sbr = const_pool.tile([P, KC, HALF], f32r)
            for c in range(KC):
                nc.scalar.dma_start(
                    out=w_sb[:, c, :], in_=w_d[c * P : (c + 1) * P, :]
                )
                # FP32r matmul operands must be produced rounded-to-FP32r.
                nc.vector.tensor_copy(out=w_sbr[:, c, :], in_=w_sb[:, c, :])

            x_tiles = {}
            o_tiles = {}

            def load_block(blk):
                # Partition p holds rows blk*512 + 4p .. 4p+3: 16 KiB
                # contiguous per partition -> large DMA descriptors.
                x_sb = xpool.tile([P, J, SIZE], f32r)
                nc.sync.dma_start(
                    out=x_sb[:],
                    in_=x_d[blk * BLK : (blk + 1) * BLK, :].rearrange(
                        "(p j) f -> p j f", j=J
                    ),
                )
                x_tiles[blk] = x_sb

            def emit_transposes(blk, j):
                # Transpose 8 chunks of [128b, 128f] -> [128f, 128b],
                # 4 chunks per PSUM bank, one Scalar-engine cast each.
                x_sb = x_tiles[blk]
                xts = []
                for h in range(2):
                    pst = tpsum.tile([P, HALF], f32r, tag="pst", name=f"pst{h}")
                    for c in range(KC):
                        k = KC * h + c
                        nc.tensor.transpose(
                            pst[:, c * P : (c + 1) * P],
                            x_sb[:, j, k * P : (k + 1) * P],
                            ident[:],
                        )
                    xt_h = xtpool.tile([P, HALF], f32r, tag="xt", name=f"xt{h}")
                    nc.scalar.copy(out=xt_h[:], in_=pst[:])
                    xts.append(xt_h)
                return xts

            def emit_mm_stage0(blk, j, xts):
                # y_half[b, :] = sum_k x_half[b, k] * W1t[k, :], h-outer so
                # the h=0 group starts as soon as its staging copy lands.
                o_sb = o_tiles[blk]
                psos = []
                for h in range(2):
                    pso = mpsum.tile(
                        [P, HALF], f32, tag="mm_psum", name=f"pso{h}"
                    )
                    for c in range(KC):
                        nc.tensor.matmul(
                            pso[:],
                            xts[h][:, c * P : (c + 1) * P],
                            w_sbr[:, c, :],
                            start=(c == 0),
                            stop=(c == KC - 1),
                        )
                    psos.append(pso)
                # Peeled stage 0: out_lo = A*y_lo + B*y_hi, out_hi =
                # C*y_lo + D*y_hi. Vector does all four multiplies straight
                # from PSUM (GpSimd cannot read PSUM); GpSimd adds. The two
                # psos[0] multiplies are emitted first so they only wait on
                # the h=0 accumulation group.
                t0 = spool.tile([P, HALF], f32, tag="t0", name="t0")
                t1 = spool.tile([P, HALF], f32, tag="t1", name="t1")
                t2 = spool.tile([P, HALF], f32, tag="t2", name="t2")
                t3 = spool.tile([P, HALF], f32, tag="t3", name="t3")
                nc.vector.tensor_mul(t0[:], psos[0][:], coef_sb[:, 0, :])
                nc.vector.tensor_mul(t2[:], psos[0][:], coef_sb[:, 2, :])
                nc.vector.tensor_mul(t1[:], psos[1][:], coef_sb[:, 1, :])
                nc.vector.tensor_mul(t3[:], psos[1][:], coef_sb[:, 3, :])
                nc.gpsimd.tensor_add(o_sb[:, j, :HALF], t0[:], t1[:])
                nc.gpsimd.tensor_add(o_sb[:, j, HALF:], t2[:], t3[:])
                # Store a j-pair as soon as it completes: rows 4p+j,4p+j+1
                # are HBM-adjacent, giving 8 KiB store descriptors on the
                # ACT queue (separate from the SP load queue).
                if blk == nblk - 1:
                    # Final block: store each j-tile as it completes so the
                    # kernel tail isn't gated on the whole block.
                    nc.scalar.dma_start(
                        out=o_d[blk * BLK : (blk + 1) * BLK, :].rearrange(
                            "(p j) f -> p j f", j=J
                        )[:, j : j + 1, :],
                        in_=o_sb[:, j : j + 1, :],
                    )
                elif j % 2 == 1:
                    nc.scalar.dma_start(
                        out=o_d[blk * BLK : (blk + 1) * BLK, :].rearrange(
                            "(p j) f -> p j f", j=J
                        )[:, j - 1 : j + 1, :],
                        in_=o_sb[:, j - 1 : j + 1, :],
                    )

            # Prefetch depth 6 blocks (matches xpool bufs).
            for blk in range(min(6, nblk)):
                load_block(blk)

            # Software pipeline: transposes of tile t+1 are emitted before
            # the matmuls of tile t, so the PE stream is
            # T(0) T(1) MM(0) T(2) MM(1) ... and never stalls on Scalar.
            tiles = [(blk, j) for blk in range(nblk) for j in range(J)]
            prev = None
            for blk, j in tiles:
                if j == 0:
                    o_tiles[blk] = opool.tile([P, J, SIZE], f32, name="o_sb")
                    if blk + 6 < nblk:
                        load_block(blk + 6)
                xts = emit_transposes(blk, j)
                if prev is not None:
                    emit_mm_stage0(*prev)
                    if prev[1] == J - 1:
                        del x_tiles[prev[0]], o_tiles[prev[0]]
                prev = (blk, j, xts)
            emit_mm_stage0(*prev)
    nc.finalize()
    return nc


def kernel(**inputs):
    global last_exec_time_ns, last_mean_exec_time_ns

    x = np.ascontiguousarray(np.asarray(inputs["x"], dtype=np.float32))
    params = [np.asarray(inputs[f"ABCD{i}"]) for i in range(M)]
    w1t = np.ascontiguousarray(_compose_w1t(params).astype(np.float32))
    coef = np.ascontiguousarray(
        np.broadcast_to(
            params[0].astype(np.float32).reshape(1, 4, HALF), (P, 4, HALF)
        )
    )

    batch = x.shape[0]
    if batch % (N_CORES * BLK) != 0:
        # Shape outside the tiled layout this kernel hardcodes - fall back
        # to a host matmul (correct, just not accelerated).
        full = _compose_w1t(params)
        y_lo = x[:, :HALF].astype(np.float64) @ full
        y_hi = x[:, HALF:].astype(np.float64) @ full
        a, b = params[0][0, 0].astype(np.float64), params[0][0, 1].astype(
            np.float64
        )
        c, dd = params[0][1, 0].astype(np.float64), params[0][1, 1].astype(
            np.float64
        )
        return np.concatenate(
            [a * y_lo + b * y_hi, c * y_lo + dd * y_hi], axis=1
        ).astype(np.float32)
    rows = batch // N_CORES

    if rows not in _nc_cache:
        _nc_cache[rows] = _build_nc(rows)
    nc = _nc_cache[rows]

    in_maps = [
        {"x": x[i * rows : (i + 1) * rows], "w": w1t, "coef": coef}
        for i in range(N_CORES)
    ]
    try:
        res = run_bass_kernel_spmd(nc, in_maps, core_ids=list(range(N_CORES)))
    except Exception:
        # Transient axon/PJRT INTERNAL errors have been observed on the
        # first attempt in a fresh process; one retry clears them.
        res = run_bass_kernel_spmd(nc, in_maps, core_ids=list(range(N_CORES)))
    last_exec_time_ns = res.exec_time_ns
    last_mean_exec_time_ns = res.mean_exec_time_ns

    return np.concatenate([r["o"] for r in res.results], axis=0)


# revision 11
# speedup vs baseline: 1.1716x; 1.1716x over previous
"""Trainium2 Bass kernel for nn_Block2x2DiagProduct (butterfly product).

Strategy:
  Stages 1..9 of the butterfly (all with block size <= 512) compose into
  blockdiag(R, R) with a single dense 512x512 matrix R shared by both
  halves (parameters are shared across blocks within each factor). The
  final stage (block size 1024) is a columnwise 2x2 butterfly:

      out[:, k]     = A[k]*y[:, k] + B[k]*y[:, 512+k]
      out[:, 512+k] = C[k]*y[:, k] + D[k]*y[:, 512+k]

  where y = x @ blockdiag(R^T, R^T). So the device kernel is two K=512
  float32r matmuls per row tile (PE) plus six columnwise multiply/adds
  (Vector + GpSimd), with Scalar staging the PE-transposed x to SBUF.

  v2 changes (trace-driven; steady state was PE-bound at 100% busy with
  DMA engines ~90%):
    - x is declared float32r end-to-end: PE transpose-mode streams f32r
      at 1.5 cycles/row vs plain fp32's 2.0 (bit-identical storage).
    - Row packing "(p j) f -> p j f": each partition holds 4 consecutive
      HBM rows, so load descriptors are 16 KiB and store descriptors
      8 KiB (vs 4 KiB), lifting per-DMA-engine throughput.
    - Stage-0 coefficients: 8 KB DMA to partition 0 + on-chip GpSimd
      partition_broadcast (was a host-broadcast 1 MiB DMA).
    - Software-pipelined PE stream: transposes of tile t+1 are emitted
      before the matmuls of tile t, so PE never waits on the Scalar
      PSUM->SBUF staging copies.
    - Matmuls grouped h-outer so the first accumulation group only waits
      on its own half's staged transpose.

  R is composed on the host in float64 (9 einsums over a 512x512
  identity). Sharding: pure data parallel - batch dim of x split across
  8 cores; R^T (1 MiB) and the stage-0 coefficients are replicated.
"""

import os
import sys

for _p in ("/opt/trn_rl_repo", "/root/.axon_site/_ro/trn_rl_repo"):
    if os.path.isdir(_p) and _p not in sys.path:
        sys.path.insert(0, _p)

import numpy as np

import concourse.bacc as bacc
import concourse.bass as bass
import concourse.mybir as mybir
from concourse.bass_utils import run_bass_kernel_spmd
from concourse.masks import make_identity
from concourse.tile import TileContext

SIZE = 1024
HALF = SIZE // 2
M = 10  # number of butterfly factors
N_CORES = 8
P = 128
KC = HALF // P  # 4 contraction chunks per half
J = 2  # rows per partition per block (8 KiB contiguous HBM per partition)
BLK = P * J  # 512 rows per block

# Results of the last device run (for the test harness).
last_exec_time_ns = None
last_mean_exec_time_ns = None

_nc_cache = {}


def _compose_w1t(params):
    """Compose butterfly stages 1..9 into W1t (512x512, f64) such that
    y_half = x_half @ W1t for each 512 half. Both halves share W1t because
    each factor's parameters are shared across its blocks."""
    w = np.eye(HALF, dtype=np.float64)
    for i in reversed(range(1, M)):
        s = SIZE >> i
        y = w.reshape(HALF, HALF // s, 2, s // 2)
        w = np.einsum(
            "ijk,bnjk->bnik", params[i].astype(np.float64), y
        ).reshape(HALF, HALF)
    return w


def _build_nc(rows):
    f32 = mybir.dt.float32
    f32r = mybir.dt.float32r
    nblk = rows // BLK

    # Bacc (not raw Bass): its finalize() pipeline splits multi-sem waits
    # into EventSemaphore instructions (HW allows 1 sync-wait per inst).
    nc = bacc.Bacc(None, target_bir_lowering=False)
    x_d = nc.dram_tensor("x", [rows, SIZE], f32r, kind="ExternalInput")
    w_d = nc.dram_tensor("w", [HALF, HALF], f32, kind="ExternalInput")
    coef_d = nc.dram_tensor("coef", [P, 4, HALF], f32, kind="ExternalInput")
    o_d = nc.dram_tensor("o", [rows, SIZE], f32, kind="ExternalOutput")

    with TileContext(nc) as tc:
        with (
            tc.tile_pool(name="const", bufs=1) as const_pool,
            tc.tile_pool(name="xin", bufs=6) as xpool,
            tc.tile_pool(name="xt", bufs=8) as xtpool,
            tc.tile_pool(name="stage", bufs=6) as spool,
            tc.tile_pool(name="osb", bufs=3) as opool,
            tc.tile_pool(name="tpsum", bufs=4, space="PSUM") as tpsum,
            tc.tile_pool(name="mpsum", bufs=4, space="PSUM") as mpsum,
        ):
            ident_f32 = const_pool.tile([P, P], f32)
            make_identity(nc, ident_f32[:])
            # GpSimd memset can't target f32r tiles, so build in f32 and
            # cast (f32r transpose needs an f32r identity operand).
            ident = const_pool.tile([P, P], f32r)
            nc.vector.tensor_copy(out=ident[:], in_=ident_f32[:])
            # PE warmup burst: the PE HAM clock-gate defaults to 1.2 GHz
            # and needs ~3.4us of sustained busy to release to 2.4 GHz.
            # The PE would otherwise sit idle until the first x load lands
            # (~12us) and then run the first blocks at half clock. These
            # no-dependency matmuls (first one doubles as the dummy
            # consuming the identity, which walrus needs so the first real
            # transpose carries a single sync-wait) run during the load
            # window and cost nothing.
            pst0 = tpsum.tile([P, P], f32r, name="pst_warm", tag="pst")
            for _ in range(36):
                nc.tensor.transpose(pst0[:], ident[:], ident[:])

            x_tiles = {}
            o_tiles = {}

            def load_block(blk):
                # Partition p holds rows blk*512 + 4p .. 4p+3: 16 KiB
                # contiguous per partition -> large DMA descriptors.
                x_sb = xpool.tile([P, J, SIZE], f32r)
                nc.sync.dma_start(
                    out=x_sb[:],
                    in_=x_d[blk * BLK : (blk + 1) * BLK, :].rearrange(
                        "(p j) f -> p j f", j=J
                    ),
                )
                x_tiles[blk] = x_sb

            def emit_transposes(blk, j):
                # Transpose 8 chunks of [128b, 128f] -> [128f, 128b],
                # 4 chunks per PSUM bank, one Scalar-engine cast each.
                x_sb = x_tiles[blk]
                xts = []
                for h in range(2):
                    pst = tpsum.tile([P, HALF], f32r, tag="pst", name=f"pst{h}")
                    for c in range(KC):
                        k = KC * h + c
                        nc.tensor.transpose(
                            pst[:, c * P : (c + 1) * P],
                            x_sb[:, j, k * P : (k + 1) * P],
                            ident[:],
                        )
                    xt_h = xtpool.tile([P, HALF], f32r, tag="xt", name=f"xt{h}")
                    nc.scalar.copy(out=xt_h[:], in_=pst[:])
                    xts.append(xt_h)
                return xts

            def emit_mm_stage0(blk, j, xts):
                # y_half[b, :] = sum_k x_half[b, k] * W1t[k, :], h-outer so
                # the h=0 group starts as soon as its staging copy lands.
                o_sb = o_tiles[blk]
                psos = []
                for h in range(2):
                    pso = mpsum.tile(
                        [P, HALF], f32, tag="mm_psum", name=f"pso{h}"
                    )
                    for c in range(KC):
                        nc.tensor.matmul(
                            pso[:],
                            xts[h][:, c * P : (c + 1) * P],
                            w_sbr[:, c, :],
                            start=(c == 0),
                            stop=(c == KC - 1),
                        )
                    psos.append(pso)
                # Peeled stage 0: out_lo = A*y_lo + B*y_hi, out_hi =
                # C*y_lo + D*y_hi. Vector does all four multiplies straight
                # from PSUM (GpSimd cannot read PSUM); GpSimd adds. The two
                # psos[0] multiplies are emitted first so they only wait on
                # the h=0 accumulation group.
                t0 = spool.tile([P, HALF], f32, tag="t0", name="t0")
                t1 = spool.tile([P, HALF], f32, tag="t1", name="t1")
                t2 = spool.tile([P, HALF], f32, tag="t2", name="t2")
                t3 = spool.tile([P, HALF], f32, tag="t3", name="t3")
                nc.vector.tensor_mul(t0[:], psos[0][:], coef_sb[:, 0, :])
                nc.vector.tensor_mul(t2[:], psos[0][:], coef_sb[:, 2, :])
                nc.vector.tensor_mul(t1[:], psos[1][:], coef_sb[:, 1, :])
                nc.vector.tensor_mul(t3[:], psos[1][:], coef_sb[:, 3, :])
                nc.gpsimd.tensor_add(o_sb[:, j, :HALF], t0[:], t1[:])
                nc.gpsimd.tensor_add(o_sb[:, j, HALF:], t2[:], t3[:])
                # Store a j-pair as soon as it completes: rows 4p+j,4p+j+1
                # are HBM-adjacent, giving 8 KiB store descriptors on the
                # ACT queue (separate from the SP load queue).
                if blk == nblk - 1:
                    # Final block: store each j-tile as it completes so the
                    # kernel tail isn't gated on the whole block.
                    nc.scalar.dma_start(
                        out=o_d[blk * BLK : (blk + 1) * BLK, :].rearrange(
                            "(p j) f -> p j f", j=J
                        )[:, j : j + 1, :],
                        in_=o_sb[:, j : j + 1, :],
                    )
                elif j % 2 == 1:
                    nc.scalar.dma_start(
                        out=o_d[blk * BLK : (blk + 1) * BLK, :].rearrange(
                            "(p j) f -> p j f", j=J
                        )[:, j - 1 : j + 1, :],
                        in_=o_sb[:, j - 1 : j + 1, :],
                    )

            # Startup-critical DMA ordering, all FIFO on the SP ring:
            # x block 0 first (gates the first transposes), then the W1t
            # chunks (gate the first matmuls), then coef (gates the first
            # stage-0 multiplies), then the prefetch flood. When w/coef sat
            # on the ACT ring they round-robined against the 6 MiB prefetch
            # and straggled to ~31us, stalling the whole stage-0 chain.
            load_block(0)
            w_sb = const_pool.tile([P, KC, HALF], f32)
            w_sbr = const_pool.tile([P, KC, HALF], f32r)
            for c in range(KC):
                nc.sync.dma_start(
                    out=w_sb[:, c, :], in_=w_d[c * P : (c + 1) * P, :]
                )
                # FP32r matmul operands must be produced rounded-to-FP32r.
                nc.vector.tensor_copy(out=w_sbr[:, c, :], in_=w_sb[:, c, :])
            coef_sb = const_pool.tile([P, 4, HALF], f32)
            nc.sync.dma_start(out=coef_sb[:], in_=coef_d[:, :, :])
            # Prefetch depth 6 blocks (matches xpool bufs).
            for blk in range(1, min(6, nblk)):
                load_block(blk)

            # Software pipeline: transposes of tile t+1 are emitted before
            # the matmuls of tile t, so the PE stream is
            # T(0) T(1) MM(0) T(2) MM(1) ... and never stalls on Scalar.
            tiles = [(blk, j) for blk in range(nblk) for j in range(J)]
            prev = None
            for blk, j in tiles:
                if j == 0:
                    o_tiles[blk] = opool.tile([P, J, SIZE], f32, name="o_sb")
                    if blk + 6 < nblk:
                        load_block(blk + 6)
                xts = emit_transposes(blk, j)
                if prev is not None:
                    emit_mm_stage0(*prev)
                    if prev[1] == J - 1:
                        del x_tiles[prev[0]], o_tiles[prev[0]]
                prev = (blk, j, xts)
            emit_mm_stage0(*prev)
    nc.finalize()
    return nc


def kernel(**inputs):
    global last_exec_time_ns, last_mean_exec_time_ns

    x = np.ascontiguousarray(np.asarray(inputs["x"], dtype=np.float32))
    params = [np.asarray(inputs[f"ABCD{i}"]) for i in range(M)]
    w1t = np.ascontiguousarray(_compose_w1t(params).astype(np.float32))
    coef = np.ascontiguousarray(
        np.broadcast_to(
            params[0].astype(np.float32).reshape(1, 4, HALF), (P, 4, HALF)
        )
    )

    batch = x.shape[0]
    if batch % (N_CORES * BLK) != 0:
        # Shape outside the tiled layout this kernel hardcodes - fall back
        # to a host matmul (correct, just not accelerated).
        full = _compose_w1t(params)
        y_lo = x[:, :HALF].astype(np.float64) @ full
        y_hi = x[:, HALF:].astype(np.float64) @ full
        a, b = params[0][0, 0].astype(np.float64), params[0][0, 1].astype(
            np.float64
        )
        c, dd = params[0][1, 0].astype(np.float64), params[0][1, 1].astype(
            np.float64
        )
        return np.concatenate(
            [a * y_lo + b * y_hi, c * y_lo + dd * y_hi], axis=1
        ).astype(np.float32)
    rows = batch // N_CORES

    if rows not in _nc_cache:
        _nc_cache[rows] = _build_nc(rows)
    nc = _nc_cache[rows]

    in_maps = [
        {"x": x[i * rows : (i + 1) * rows], "w": w1t, "coef": coef}
        for i in range(N_CORES)
    ]
    try:
        res = run_bass_kernel_spmd(nc, in_maps, core_ids=list(range(N_CORES)))
    except Exception:
        # Transient axon/PJRT INTERNAL errors have been observed on the
        # first attempt in a fresh process; one retry clears them.
        res = run_bass_kernel_spmd(nc, in_maps, core_ids=list(range(N_CORES)))
    last_exec_time_ns = res.exec_time_ns
    last_mean_exec_time_ns = res.mean_exec_time_ns

    return np.concatenate([r["o"] for r in res.results], axis=0)
